# revision 15
# baseline (speedup 1.0000x reference)
"""Two-layer GCN (PyG GCNConv x2 + ReLU) on 8 Trainium2 NeuronCores.

Strategy (graph/data parallel, per the destination-partitioned sharding):
  - Nodes are row-sharded across 8 cores (6250 real + pad -> 6272 per core).
  - Edges (plus one self-edge per node, which realises the GCN self-loop
    term exactly) are partitioned by destination owner and grouped by
    destination tile (128 dst nodes), then by source-row region
    (lo: table row < 32768, hi: >= 32768) so gather indices fit in int16.
  - Per layer on each core:  h^T = W^T @ x^T on PE;  g^T = h^T * dinv
    (the symmetric norm dinv[src]*dinv[dst] folds into per-node scaling);
    g rows are written to DRAM and AllGather'ed into a replicated node
    table;  per-edge source rows are fetched with SWDGE dma_gather;  the
    segment-sum by destination is a PE matmul against a one-hot S matrix
    built on the vector engine (iota == dstid);  epilogue scales by
    dinv[dst], adds bias and applies ReLU.
  - fp16 operands with fp32 PSUM accumulation.
"""

import math
import os
import sys

import numpy as np

for _p in ("/opt/trn_rl_repo", "/root/.axon_site/_ro/trn_rl_repo"):
    if os.path.isdir(_p) and _p not in sys.path:
        sys.path.append(_p)

import concourse.bacc as bacc
import concourse.bass as bass
import concourse.mybir as mybir
import concourse.tile as tile
from concourse.bass_utils import run_bass_kernel_spmd

# Problem constants (hardcoded per harness contract).
N, E, IN, HID, OUT = 50000, 800000, 128, 128, 64
NCORES = 8
NPC_REAL = N // NCORES          # 6250
TILES = 49
NPC = TILES * 128               # 6272 padded nodes per core
R = NCORES * NPC                # 50176 table rows
LO = 32768                      # int16-reachable row count
WV = 32                         # gather wave size in chunks (128 slots each)


def default_cfg():
    return dict(N=N, E=E, IN=IN, HID=HID, OUT=OUT, NCORES=NCORES,
                NPC_REAL=NPC_REAL, TILES=TILES, NPC=NPC, R=R, LO=LO, WV=WV)

F16 = mybir.dt.float16
F32 = mybir.dt.float32
NPF16 = np.float16

_ts = bass.ts


def _preprocess(edge_index: np.ndarray, cfg=None):
    """Partition/sort/pad edges; build per-core gather-index and dst-id
    arrays plus the shared static chunk schedule."""
    g = cfg or default_cfg()
    N, NCORES, NPC_REAL, TILES, NPC, LO = (
        g["N"], g["NCORES"], g["NPC_REAL"], g["TILES"], g["NPC"], g["LO"])
    src = np.asarray(edge_index[0], np.int64)
    dst = np.asarray(edge_index[1], np.int64)
    deg = np.bincount(dst, minlength=N).astype(np.float64) + 1.0

    selfn = np.arange(N, dtype=np.int64)
    src_all = np.concatenate([src, selfn])
    dst_all = np.concatenate([dst, selfn])

    owner = dst_all // NPC_REAL
    dst_local = dst_all % NPC_REAL
    src_row = (src_all // NPC_REAL) * NPC + (src_all % NPC_REAL)
    tile_id = dst_local // 128
    intile = dst_local % 128
    region = (src_row >= LO).astype(np.int64)

    cnt = np.zeros((NCORES, TILES, 2), np.int64)
    np.add.at(cnt, (owner, tile_id, region), 1)
    K = np.ceil(cnt.max(axis=0) / 128).astype(np.int64)   # [TILES, 2]
    K_lo, K_hi = K[:, 0].copy(), K[:, 1].copy()
    C_lo, C_hi = int(K_lo.sum()), int(K_hi.sum())
    C = C_lo + C_hi
    LB = np.concatenate([[0], np.cumsum(K_lo)[:-1]]).astype(np.int64)
    HB = (C_lo + np.concatenate([[0], np.cumsum(K_hi)[:-1]])).astype(np.int64)

    # slot assignment: sort edges by (owner, region, tile); position within
    # each (owner, tile, region) group via cumulative count
    gid = (owner * TILES + tile_id) * 2 + region
    order = np.argsort(gid, kind="stable")
    gs = gid[order]
    starts = np.concatenate([[0], np.flatnonzero(np.diff(gs)) + 1])
    group_of = np.searchsorted(starts, np.arange(len(gs)), side="right") - 1
    pos = np.arange(len(gs)) - starts[group_of]

    base_chunk = np.where(region == 0, LB[tile_id], HB[tile_id])
    slot = np.empty(len(gs), np.int64)
    slot[order] = base_chunk[order] * 128 + pos

    nslots = C * 128
    idx16 = np.zeros((NCORES, nslots), np.int16)
    dstid = np.full((NCORES, nslots), -1.0, np.float32)
    idx16[owner, slot] = (src_row - region * LO).astype(np.int16)
    dstid[owner, slot] = intile

    # wrapped+replicated gather index tile [128, C*8] and dst-id tile [128, C]
    idx_t = idx16.reshape(NCORES, C * 8, 16).transpose(0, 2, 1)     # [8,16,C*8]
    idx_t = np.tile(idx_t, (1, 8, 1)).copy()                        # [8,128,C*8]
    dstid_t = dstid.reshape(NCORES, C, 128).transpose(0, 2, 1).astype(np.float32).copy()

    return dict(deg=deg, K_lo=K_lo, K_hi=K_hi, C_lo=C_lo, C_hi=C_hi, C=C,
                LB=LB, HB=HB, idx_t=idx_t, dstid_t=dstid_t)


def _waves(n_chunks: int, chunk0: int, wv: int = WV):
    out = []
    c = 0
    while c < n_chunks:
        n = min(wv, n_chunks - c)
        out.append((chunk0 + c, n))
        c += n
    return out


def _build_program(meta, cfg=None):
    g = cfg or default_cfg()
    IN, HID, OUT = g["IN"], g["HID"], g["OUT"]
    NCORES, TILES, NPC, R, LO, WV = (g["NCORES"], g["TILES"], g["NPC"],
                                     g["R"], g["LO"], g["WV"])
    stage = g.get("stage", "full")   # "ag" | "gather" | "full"
    K_lo, K_hi = meta["K_lo"], meta["K_hi"]
    C_lo, C_hi, C = meta["C_lo"], meta["C_hi"], meta["C"]
    LB, HB = meta["LB"], meta["HB"]

    nc = bacc.Bacc("TRN2", target_bir_lowering=False, debug=False,
                   num_devices=NCORES)

    # ---- I/O ----
    xT_d = nc.dram_tensor("xT", [IN, NPC], F16, kind="ExternalInput")
    w1_d = nc.dram_tensor("W1", [IN, HID], F16, kind="ExternalInput")
    w2_d = nc.dram_tensor("W2", [HID, OUT], F16, kind="ExternalInput")
    b1_d = nc.dram_tensor("b1c", [HID, 1], F32, kind="ExternalInput")
    b2_d = nc.dram_tensor("b2c", [OUT, 1], F32, kind="ExternalInput")
    deg_d = nc.dram_tensor("degrep", [128, NPC], F32, kind="ExternalInput")
    iota_d = nc.dram_tensor("iota", [128, 128], F16, kind="ExternalInput")
    ident_d = nc.dram_tensor("ident", [128, 128], F16, kind="ExternalInput")
    idx_d = nc.dram_tensor("idxt", [128, C * 8], mybir.dt.int16,
                           kind="ExternalInput")
    dstid_d = nc.dram_tensor("dstidt", [128, C], F32, kind="ExternalInput")
    out_d = nc.dram_tensor("outT", [OUT, NPC], F32, kind="ExternalOutput")

    # ---- internal DRAM (collective bounce + replicated tables) ----
    gdram = [nc.dram_tensor(f"gdram{l}", [NPC, 128], F16) for l in (1, 2)]
    table = [nc.dram_tensor(f"table{l}", [R, 128], F16, addr_space="Shared")
             for l in (1, 2)]

    rg = [list(range(NCORES))]

    with tile.TileContext(nc) as tc:
        with (
            tc.tile_pool(name="const", bufs=1) as constp,
            tc.tile_pool(name="big", bufs=3) as bigp,
            tc.tile_pool(name="glo", bufs=2) as glop,
            tc.tile_pool(name="ghi", bufs=2) as ghip,
            tc.tile_pool(name="s", bufs=6) as sp,
            tc.tile_pool(name="tmp", bufs=4) as tmpp,
            tc.tile_pool(name="pmm", bufs=2, space="PSUM") as pmm,
            tc.tile_pool(name="ptr", bufs=2, space="PSUM") as ptr,
            tc.tile_pool(name="psc", bufs=4, space="PSUM") as psc,
        ):
            # ---- constants / inputs to SBUF ----
            xT = bigp.tile([128, NPC], F16, tag="bigf16")
            nc.sync.dma_start(xT[:IN, :], xT_d[:, :])
            w1 = constp.tile([IN, HID], F16, tag="w1")
            nc.sync.dma_start(w1[:], w1_d[:, :])
            w2 = constp.tile([HID, OUT], F16, tag="w2")
            nc.sync.dma_start(w2[:], w2_d[:, :])
            b1 = constp.tile([HID, 1], F32, tag="b1")
            nc.sync.dma_start(b1[:], b1_d[:, :])
            b2 = constp.tile([OUT, 1], F32, tag="b2")
            nc.sync.dma_start(b2[:], b2_d[:, :])
            iota = constp.tile([128, 128], F16, tag="iota")
            nc.sync.dma_start(iota[:], iota_d[:, :])
            ident = constp.tile([128, 128], F16, tag="ident")
            nc.sync.dma_start(ident[:], ident_d[:, :])
            idxt = constp.tile([128, C * 8], mybir.dt.int16, tag="idxt")
            nc.sync.dma_start(idxt[:], idx_d[:, :])
            dstid = constp.tile([128, C], F32, tag="dstid")
            nc.sync.dma_start(dstid[:], dstid_d[:, :])

            # dinv_rep = sqrt(1/deg), partition-replicated [128, NPC] f32
            dinv = constp.tile([128, NPC], F32, tag="dinv")
            nc.sync.dma_start(dinv[:], deg_d[:, :])
            nc.vector.reciprocal(dinv[:], dinv[:])
            nc.scalar.sqrt(dinv[:], dinv[:])

            relu1 = None

            for layer in (0, 1):
                Fdim = HID if layer == 0 else OUT
                W = w1 if layer == 0 else w2
                bcol = b1 if layer == 0 else b2
                rhs_in = xT if layer == 0 else relu1

                # ---- 1. h^T = W^T @ rhs ; g^T = h^T * dinv (fp16) ----
                gT = bigp.tile([128, NPC], F16, tag="bigf16")
                nmm = math.ceil(NPC / 512)
                for i in range(nmm):
                    w_ = min(512, NPC - i * 512)
                    sl = slice(i * 512, i * 512 + w_)
                    ps = pmm.tile([128, 512], F32, tag="pmm")
                    nc.tensor.matmul(ps[:Fdim, :w_], W[:, :Fdim],
                                     rhs_in[:128, sl],
                                     start=True, stop=True)
                    nc.vector.scalar_tensor_tensor(
                        gT[:Fdim, sl], ps[:Fdim, :w_], 0.0,
                        dinv[:Fdim, sl],
                        mybir.AluOpType.bypass, mybir.AluOpType.mult)

                # ---- 2. transpose per dst tile into row-major staged ----
                staged = bigp.tile([128, NPC], F16, tag="bigf16")
                if Fdim < 128:
                    nc.vector.memset(staged[:], 0.0)
                for t in range(TILES):
                    pt = ptr.tile([128, Fdim], F16, tag="ptr")
                    nc.tensor.transpose(pt[:, :], gT[:Fdim, _ts(t, 128)],
                                        ident[:Fdim, :Fdim])
                    nc.vector.tensor_copy(staged[:, _ts(t, 128)][:, :Fdim],
                                          pt[:, :])

                # ---- 3. staged -> DRAM rows; AllGather into table ----
                gview = gdram[layer].ap().rearrange("(t p) f -> p t f", p=128)
                sview = staged[:].rearrange("p (t f) -> p t f", f=128)
                nc.sync.dma_start(gview, sview)
                nc.gpsimd.collective_compute(
                    "AllGather", mybir.AluOpType.bypass, replica_groups=rg,
                    ins=[gdram[layer].ap()], outs=[table[layer].ap()])

                # ---- 4. gather waves + one-hot scatter matmuls ----
                lo_waves = _waves(C_lo, 0, WV)
                hi_waves = _waves(C_hi, C_lo, WV)
                wave_tiles = {}

                def ensure_wave(rgn, wi, _wt=wave_tiles, _lw=lo_waves,
                                _hw=hi_waves, _ly=layer):
                    key = (rgn, wi)
                    if key in _wt:
                        return _wt[key]
                    c0, n = (_lw if rgn == 0 else _hw)[wi]
                    pool = glop if rgn == 0 else ghip
                    g = pool.tile([128, WV, 128], F16,
                                  tag="glo" if rgn == 0 else "ghi")
                    src = (table[_ly][0:LO, :] if rgn == 0
                           else table[_ly][LO:R, :])
                    nc.gpsimd.dma_gather(
                        g[:, :n, :], src, idxt[:, c0 * 8:(c0 + n) * 8],
                        n * 128, n * 128, 128, single_packet=False)
                    _wt[key] = g
                    return g

                target = None
                if layer == 0:
                    relu1 = bigp.tile([128, NPC], F16, tag="bigf16")
                    target = relu1
                else:
                    outs = bigp.tile([OUT, NPC], F32, tag="bigf32")
                    target = outs

                if stage == "ag":
                    nc.vector.memset(target[:Fdim, :], 0.0)
                    continue
                if stage == "gather":
                    for wi in range(len(lo_waves)):
                        ensure_wave(0, wi)
                    for wi in range(len(hi_waves)):
                        ensure_wave(1, wi)
                    nc.vector.memset(target[:Fdim, :], 0.0)
                    continue

                for t in range(TILES):
                    nchunks = int(K_lo[t] + K_hi[t])
                    if nchunks == 0:
                        nc.vector.memset(target[:Fdim, _ts(t, 128)], 0.0)
                        continue
                    pscat = psc.tile([Fdim, 128], F32, tag="psc")
                    ci = 0
                    for rgn, Kr, Bs, off in ((0, K_lo, LB, 0),
                                             (1, K_hi, HB, C_lo)):
                        for k in range(int(Kr[t])):
                            ch = int(Bs[t]) + k          # global chunk id
                            rel = ch - off               # chunk id in region
                            g = ensure_wave(rgn, rel // WV)
                            pos = rel % WV
                            if stage == "consts":
                                s_t = iota
                            else:
                                s_t = sp.tile([128, 128], F16, tag="s")
                                nc.vector.tensor_scalar(
                                    s_t[:], iota[:], dstid[:, ch:ch + 1], None,
                                    mybir.AluOpType.is_equal)
                            nc.tensor.matmul(
                                pscat[:Fdim, :], g[:, pos, :Fdim], s_t[:],
                                start=(ci == 0), stop=(ci == nchunks - 1))
                            ci += 1

                    # ---- 5. epilogue: *dinv[dst], +bias, ReLU ----
                    if stage == "noepi":
                        nc.vector.memset(target[:Fdim, _ts(t, 128)], 0.0)
                        continue
                    tmp = tmpp.tile([Fdim, 128], F32, tag="tmp")
                    nc.vector.scalar_tensor_tensor(
                        tmp[:Fdim, :], pscat[:Fdim, :], 0.0,
                        dinv[:Fdim, _ts(t, 128)],
                        mybir.AluOpType.bypass, mybir.AluOpType.mult)
                    nc.vector.tensor_scalar(
                        target[:Fdim, _ts(t, 128)], tmp[:Fdim, :],
                        bcol[:Fdim, :], 0.0,
                        mybir.AluOpType.add, mybir.AluOpType.max)

            nc.sync.dma_start(out_d[:, :], target[:OUT, :])

    nc.compile()
    return nc


def _host_inputs(inputs, meta, cfg=None):
    g = cfg or default_cfg()
    N, IN, HID, OUT = g["N"], g["IN"], g["HID"], g["OUT"]
    NCORES, NPC_REAL, NPC = g["NCORES"], g["NPC_REAL"], g["NPC"]
    x = np.asarray(inputs["x"], np.float32)
    W1 = np.asarray(inputs["W1"], np.float32)
    b1 = np.asarray(inputs["b1"], np.float32)
    W2 = np.asarray(inputs["W2"], np.float32)
    b2 = np.asarray(inputs["b2"], np.float32)
    deg = meta["deg"]

    iota = np.tile(np.arange(128, dtype=NPF16)[None, :], (128, 1))
    ident = np.eye(128, dtype=NPF16)
    w1c = W1.astype(NPF16)
    w2c = np.zeros((HID, OUT), NPF16)
    w2c[:, :] = W2.astype(NPF16)
    b1c = b1.reshape(HID, 1).astype(np.float32)
    b2c = b2.reshape(OUT, 1).astype(np.float32)

    in_maps = []
    for c in range(NCORES):
        xs = np.zeros((NPC, IN), np.float32)
        xs[:NPC_REAL] = x[c * NPC_REAL:(c + 1) * NPC_REAL]
        xT = np.ascontiguousarray(xs.T).astype(NPF16)

        node = np.arange(NPC) + c * NPC_REAL
        degs = np.ones(NPC, np.float32)
        degs[:NPC_REAL] = deg[node[:NPC_REAL]]
        degrep = np.tile(degs[None, :], (128, 1)).astype(np.float32)

        in_maps.append({
            "xT": xT, "W1": w1c, "W2": w2c, "b1c": b1c, "b2c": b2c,
            "degrep": degrep, "iota": iota, "ident": ident,
            "idxt": meta["idx_t"][c], "dstidt": meta["dstid_t"][c],
        })
    return in_maps


def kernel(**inputs) -> np.ndarray:
    meta = _preprocess(np.asarray(inputs["edge_index"]))
    nc = _build_program(meta)
    in_maps = _host_inputs(inputs, meta)
    res = run_bass_kernel_spmd(nc, in_maps, list(range(NCORES)))
    out = np.empty((N, OUT), np.float32)
    for c in range(NCORES):
        out[c * NPC_REAL:(c + 1) * NPC_REAL] = \
            res.results[c]["outT"][:, :NPC_REAL].T
    return out


# revision 16
# speedup vs baseline: 1.1545x; 1.1545x over previous
"""Two-layer GCN (PyG GCNConv x2 + ReLU) on 8 Trainium2 NeuronCores.

Strategy (graph/data parallel, per the destination-partitioned sharding):
  - Nodes are row-sharded across 8 cores (6250 real + pad -> 6272 per core).
  - Edges (plus one self-edge per node, which realises the GCN self-loop
    term exactly) are partitioned by destination owner and grouped by
    destination tile (128 dst nodes), then by source-row region
    (lo: table row < 32768, hi: >= 32768) so gather indices fit in int16.
  - Per layer on each core:  h^T = W^T @ x^T on PE;  g^T = h^T * dinv
    (the symmetric norm dinv[src]*dinv[dst] folds into per-node scaling);
    g rows are written to DRAM and AllGather'ed into a replicated node
    table;  per-edge source rows are fetched with SWDGE dma_gather;  the
    segment-sum by destination is a PE matmul against a one-hot S matrix
    built on the vector engine (iota == dstid);  epilogue scales by
    dinv[dst], adds bias and applies ReLU.
  - fp16 operands with fp32 PSUM accumulation.
"""

import math
import os
import sys

import numpy as np

for _p in ("/opt/trn_rl_repo", "/root/.axon_site/_ro/trn_rl_repo"):
    if os.path.isdir(_p) and _p not in sys.path:
        sys.path.append(_p)

import concourse.bacc as bacc
import concourse.bass as bass
import concourse.mybir as mybir
import concourse.tile as tile
from concourse.bass_utils import run_bass_kernel_spmd

# Problem constants (hardcoded per harness contract).
N, E, IN, HID, OUT = 50000, 800000, 128, 128, 64
NCORES = 8
NPC_REAL = N // NCORES          # 6250
TILES = 49
NPC = TILES * 128               # 6272 padded nodes per core
R = NCORES * NPC                # 50176 table rows
LO = 32768                      # int16-reachable row count
WV = 32                         # gather wave size in chunks (128 slots each)


def default_cfg():
    return dict(N=N, E=E, IN=IN, HID=HID, OUT=OUT, NCORES=NCORES,
                NPC_REAL=NPC_REAL, TILES=TILES, NPC=NPC, R=R, LO=LO, WV=WV)

F16 = mybir.dt.float16
F32 = mybir.dt.float32
NPF16 = np.float16

_ts = bass.ts


def _preprocess(edge_index: np.ndarray, cfg=None):
    """Partition/sort/pad edges; build per-core gather-index and dst-id
    arrays plus the shared static chunk schedule."""
    g = cfg or default_cfg()
    N, NCORES, NPC_REAL, TILES, NPC, LO = (
        g["N"], g["NCORES"], g["NPC_REAL"], g["TILES"], g["NPC"], g["LO"])
    src = np.asarray(edge_index[0], np.int64)
    dst = np.asarray(edge_index[1], np.int64)
    deg = np.bincount(dst, minlength=N).astype(np.float64) + 1.0

    selfn = np.arange(N, dtype=np.int64)
    src_all = np.concatenate([src, selfn])
    dst_all = np.concatenate([dst, selfn])

    owner = dst_all // NPC_REAL
    dst_local = dst_all % NPC_REAL
    src_row = (src_all // NPC_REAL) * NPC + (src_all % NPC_REAL)
    tile_id = dst_local // 128
    intile = dst_local % 128
    region = (src_row >= LO).astype(np.int64)

    cnt = np.zeros((NCORES, TILES, 2), np.int64)
    np.add.at(cnt, (owner, tile_id, region), 1)
    K = np.ceil(cnt.max(axis=0) / 128).astype(np.int64)   # [TILES, 2]
    K_lo, K_hi = K[:, 0].copy(), K[:, 1].copy()
    C_lo, C_hi = int(K_lo.sum()), int(K_hi.sum())
    C = C_lo + C_hi
    LB = np.concatenate([[0], np.cumsum(K_lo)[:-1]]).astype(np.int64)
    HB = (C_lo + np.concatenate([[0], np.cumsum(K_hi)[:-1]])).astype(np.int64)

    # slot assignment: sort edges by (owner, region, tile); position within
    # each (owner, tile, region) group via cumulative count
    gid = (owner * TILES + tile_id) * 2 + region
    order = np.argsort(gid, kind="stable")
    gs = gid[order]
    starts = np.concatenate([[0], np.flatnonzero(np.diff(gs)) + 1])
    group_of = np.searchsorted(starts, np.arange(len(gs)), side="right") - 1
    pos = np.arange(len(gs)) - starts[group_of]

    base_chunk = np.where(region == 0, LB[tile_id], HB[tile_id])
    slot = np.empty(len(gs), np.int64)
    slot[order] = base_chunk[order] * 128 + pos

    nslots = C * 128
    idx16 = np.zeros((NCORES, nslots), np.int16)
    dstid = np.full((NCORES, nslots), -1.0, np.float32)
    idx16[owner, slot] = (src_row - region * LO).astype(np.int16)
    dstid[owner, slot] = intile

    # wrapped+replicated gather index tile [128, C*8] and dst-id tile [128, C]
    idx_t = idx16.reshape(NCORES, C * 8, 16).transpose(0, 2, 1)     # [8,16,C*8]
    idx_t = np.tile(idx_t, (1, 8, 1)).copy()                        # [8,128,C*8]
    dstid_t = dstid.reshape(NCORES, C, 128).transpose(0, 2, 1).astype(np.float32).copy()

    return dict(deg=deg, K_lo=K_lo, K_hi=K_hi, C_lo=C_lo, C_hi=C_hi, C=C,
                LB=LB, HB=HB, idx_t=idx_t, dstid_t=dstid_t)


def _waves(n_chunks: int, chunk0: int, wv: int = WV):
    out = []
    c = 0
    while c < n_chunks:
        n = min(wv, n_chunks - c)
        out.append((chunk0 + c, n))
        c += n
    return out


def _build_program(meta, cfg=None):
    g = cfg or default_cfg()
    IN, HID, OUT = g["IN"], g["HID"], g["OUT"]
    NCORES, TILES, NPC, R, LO, WV = (g["NCORES"], g["TILES"], g["NPC"],
                                     g["R"], g["LO"], g["WV"])
    stage = g.get("stage", "full")   # "ag" | "gather" | "full"
    K_lo, K_hi = meta["K_lo"], meta["K_hi"]
    C_lo, C_hi, C = meta["C_lo"], meta["C_hi"], meta["C"]
    LB, HB = meta["LB"], meta["HB"]

    nc = bacc.Bacc("TRN2", target_bir_lowering=False, debug=False,
                   num_devices=NCORES, num_swdge_queues=4)

    # ---- I/O ----
    xT_d = nc.dram_tensor("xT", [IN, NPC], F16, kind="ExternalInput")
    w1_d = nc.dram_tensor("W1", [IN, HID], F16, kind="ExternalInput")
    w2_d = nc.dram_tensor("W2", [HID, OUT], F16, kind="ExternalInput")
    b1_d = nc.dram_tensor("b1c", [HID, 1], F32, kind="ExternalInput")
    b2_d = nc.dram_tensor("b2c", [OUT, 1], F32, kind="ExternalInput")
    deg_d = nc.dram_tensor("degrep", [128, NPC], F32, kind="ExternalInput")
    iota_d = nc.dram_tensor("iota", [128, 128], F16, kind="ExternalInput")
    ident_d = nc.dram_tensor("ident", [128, 128], F16, kind="ExternalInput")
    idx_d = nc.dram_tensor("idxt", [128, C * 8], mybir.dt.int16,
                           kind="ExternalInput")
    dstid_d = nc.dram_tensor("dstidt", [128, C], F32, kind="ExternalInput")
    out_d = nc.dram_tensor("outT", [OUT, NPC], F32, kind="ExternalOutput")

    # ---- internal DRAM (collective bounce + replicated tables) ----
    gdram = [nc.dram_tensor(f"gdram{l}", [NPC, 128], F16) for l in (1, 2)]
    table = [nc.dram_tensor(f"table{l}", [R, 128], F16, addr_space="Shared")
             for l in (1, 2)]

    rg = [list(range(NCORES))]

    with tile.TileContext(nc) as tc:
        with (
            tc.tile_pool(name="const", bufs=1) as constp,
            tc.tile_pool(name="big", bufs=3) as bigp,
            tc.tile_pool(name="glo", bufs=2) as glop,
            tc.tile_pool(name="ghi", bufs=2) as ghip,
            tc.tile_pool(name="s", bufs=6) as sp,
            tc.tile_pool(name="tmp", bufs=4) as tmpp,
            tc.tile_pool(name="pmm", bufs=2, space="PSUM") as pmm,
            tc.tile_pool(name="ptr", bufs=2, space="PSUM") as ptr,
            tc.tile_pool(name="psc", bufs=4, space="PSUM") as psc,
        ):
            # ---- constants / inputs to SBUF ----
            xT = bigp.tile([128, NPC], F16, tag="bigf16")
            nc.sync.dma_start(xT[:IN, :], xT_d[:, :])
            w1 = constp.tile([IN, HID], F16, tag="w1")
            nc.sync.dma_start(w1[:], w1_d[:, :])
            w2 = constp.tile([HID, OUT], F16, tag="w2")
            nc.sync.dma_start(w2[:], w2_d[:, :])
            b1 = constp.tile([HID, 1], F32, tag="b1")
            nc.sync.dma_start(b1[:], b1_d[:, :])
            b2 = constp.tile([OUT, 1], F32, tag="b2")
            nc.sync.dma_start(b2[:], b2_d[:, :])
            iota = constp.tile([128, 128], F16, tag="iota")
            nc.sync.dma_start(iota[:], iota_d[:, :])
            ident = constp.tile([128, 128], F16, tag="ident")
            nc.sync.dma_start(ident[:], ident_d[:, :])
            idxt = constp.tile([128, C * 8], mybir.dt.int16, tag="idxt")
            nc.sync.dma_start(idxt[:], idx_d[:, :])
            dstid = constp.tile([128, C], F32, tag="dstid")
            nc.sync.dma_start(dstid[:], dstid_d[:, :])

            # dinv_rep = sqrt(1/deg), partition-replicated [128, NPC] f32
            dinv = constp.tile([128, NPC], F32, tag="dinv")
            nc.sync.dma_start(dinv[:], deg_d[:, :])
            nc.vector.reciprocal(dinv[:], dinv[:])
            nc.scalar.sqrt(dinv[:], dinv[:])

            relu1 = None

            for layer in (0, 1):
                Fdim = HID if layer == 0 else OUT
                W = w1 if layer == 0 else w2
                bcol = b1 if layer == 0 else b2
                rhs_in = xT if layer == 0 else relu1

                # ---- 1. h^T = W^T @ rhs ; g^T = h^T * dinv (fp16) ----
                gT = bigp.tile([128, NPC], F16, tag="bigf16")
                nmm = math.ceil(NPC / 512)
                for i in range(nmm):
                    w_ = min(512, NPC - i * 512)
                    sl = slice(i * 512, i * 512 + w_)
                    ps = pmm.tile([128, 512], F32, tag="pmm")
                    nc.tensor.matmul(ps[:Fdim, :w_], W[:, :Fdim],
                                     rhs_in[:128, sl],
                                     start=True, stop=True)
                    nc.vector.scalar_tensor_tensor(
                        gT[:Fdim, sl], ps[:Fdim, :w_], 0.0,
                        dinv[:Fdim, sl],
                        mybir.AluOpType.bypass, mybir.AluOpType.mult)

                # ---- 2. transpose per dst tile into row-major staged ----
                staged = bigp.tile([128, NPC], F16, tag="bigf16")
                if Fdim < 128:
                    nc.vector.memset(staged[:], 0.0)
                for t in range(TILES):
                    pt = ptr.tile([128, Fdim], F16, tag="ptr")
                    nc.tensor.transpose(pt[:, :], gT[:Fdim, _ts(t, 128)],
                                        ident[:Fdim, :Fdim])
                    nc.vector.tensor_copy(staged[:, _ts(t, 128)][:, :Fdim],
                                          pt[:, :])

                # ---- 3. staged -> DRAM rows; AllGather into table ----
                gview = gdram[layer].ap().rearrange("(t p) f -> p t f", p=128)
                sview = staged[:].rearrange("p (t f) -> p t f", f=128)
                nc.sync.dma_start(gview, sview)
                nc.gpsimd.collective_compute(
                    "AllGather", mybir.AluOpType.bypass, replica_groups=rg,
                    ins=[gdram[layer].ap()], outs=[table[layer].ap()])

                # ---- 4. gather waves + one-hot scatter matmuls ----
                lo_waves = _waves(C_lo, 0, WV)
                hi_waves = _waves(C_hi, C_lo, WV)
                wave_tiles = {}

                def ensure_wave(rgn, wi, _wt=wave_tiles, _lw=lo_waves,
                                _hw=hi_waves, _ly=layer):
                    key = (rgn, wi)
                    if key in _wt:
                        return _wt[key]
                    c0, n = (_lw if rgn == 0 else _hw)[wi]
                    pool = glop if rgn == 0 else ghip
                    g = pool.tile([128, WV, 128], F16,
                                  tag="glo" if rgn == 0 else "ghi")
                    src = (table[_ly][0:LO, :] if rgn == 0
                           else table[_ly][LO:R, :])
                    qn = ensure_wave.q[0]
                    ensure_wave.q[0] = (qn + 1) % 4
                    nc.gpsimd.dma_gather(
                        g[:, :n, :], src, idxt[:, c0 * 8:(c0 + n) * 8],
                        n * 128, n * 128, 128, single_packet=False,
                        queue_num=qn)
                    _wt[key] = g
                    return g

                ensure_wave.q = [0]

                target = None
                if layer == 0:
                    relu1 = bigp.tile([128, NPC], F16, tag="bigf16")
                    target = relu1
                else:
                    outs = bigp.tile([OUT, NPC], F32, tag="bigf32")
                    target = outs

                if stage == "ag":
                    nc.vector.memset(target[:Fdim, :], 0.0)
                    continue
                if stage == "gather":
                    for wi in range(len(lo_waves)):
                        ensure_wave(0, wi)
                    for wi in range(len(hi_waves)):
                        ensure_wave(1, wi)
                    nc.vector.memset(target[:Fdim, :], 0.0)
                    continue

                for t in range(TILES):
                    nchunks = int(K_lo[t] + K_hi[t])
                    if nchunks == 0:
                        nc.vector.memset(target[:Fdim, _ts(t, 128)], 0.0)
                        continue
                    pscat = psc.tile([Fdim, 128], F32, tag="psc")
                    ci = 0
                    for rgn, Kr, Bs, off in ((0, K_lo, LB, 0),
                                             (1, K_hi, HB, C_lo)):
                        for k in range(int(Kr[t])):
                            ch = int(Bs[t]) + k          # global chunk id
                            rel = ch - off               # chunk id in region
                            g = ensure_wave(rgn, rel // WV)
                            pos = rel % WV
                            if stage == "consts":
                                s_t = iota
                            else:
                                s_t = sp.tile([128, 128], F16, tag="s")
                                nc.vector.tensor_scalar(
                                    s_t[:], iota[:], dstid[:, ch:ch + 1], None,
                                    mybir.AluOpType.is_equal)
                            nc.tensor.matmul(
                                pscat[:Fdim, :], g[:, pos, :Fdim], s_t[:],
                                start=(ci == 0), stop=(ci == nchunks - 1))
                            ci += 1

                    # ---- 5. epilogue: *dinv[dst], +bias, ReLU ----
                    if stage == "noepi":
                        nc.vector.memset(target[:Fdim, _ts(t, 128)], 0.0)
                        continue
                    tmp = tmpp.tile([Fdim, 128], F32, tag="tmp")
                    nc.vector.scalar_tensor_tensor(
                        tmp[:Fdim, :], pscat[:Fdim, :], 0.0,
                        dinv[:Fdim, _ts(t, 128)],
                        mybir.AluOpType.bypass, mybir.AluOpType.mult)
                    nc.vector.tensor_scalar(
                        target[:Fdim, _ts(t, 128)], tmp[:Fdim, :],
                        bcol[:Fdim, :], 0.0,
                        mybir.AluOpType.add, mybir.AluOpType.max)

            nc.sync.dma_start(out_d[:, :], target[:OUT, :])

    nc.compile()
    return nc


def _host_inputs(inputs, meta, cfg=None):
    g = cfg or default_cfg()
    N, IN, HID, OUT = g["N"], g["IN"], g["HID"], g["OUT"]
    NCORES, NPC_REAL, NPC = g["NCORES"], g["NPC_REAL"], g["NPC"]
    x = np.asarray(inputs["x"], np.float32)
    W1 = np.asarray(inputs["W1"], np.float32)
    b1 = np.asarray(inputs["b1"], np.float32)
    W2 = np.asarray(inputs["W2"], np.float32)
    b2 = np.asarray(inputs["b2"], np.float32)
    deg = meta["deg"]

    iota = np.tile(np.arange(128, dtype=NPF16)[None, :], (128, 1))
    ident = np.eye(128, dtype=NPF16)
    w1c = W1.astype(NPF16)
    w2c = np.zeros((HID, OUT), NPF16)
    w2c[:, :] = W2.astype(NPF16)
    b1c = b1.reshape(HID, 1).astype(np.float32)
    b2c = b2.reshape(OUT, 1).astype(np.float32)

    in_maps = []
    for c in range(NCORES):
        xs = np.zeros((NPC, IN), np.float32)
        xs[:NPC_REAL] = x[c * NPC_REAL:(c + 1) * NPC_REAL]
        xT = np.ascontiguousarray(xs.T).astype(NPF16)

        node = np.arange(NPC) + c * NPC_REAL
        degs = np.ones(NPC, np.float32)
        degs[:NPC_REAL] = deg[node[:NPC_REAL]]
        degrep = np.tile(degs[None, :], (128, 1)).astype(np.float32)

        in_maps.append({
            "xT": xT, "W1": w1c, "W2": w2c, "b1c": b1c, "b2c": b2c,
            "degrep": degrep, "iota": iota, "ident": ident,
            "idxt": meta["idx_t"][c], "dstidt": meta["dstid_t"][c],
        })
    return in_maps


def kernel(**inputs) -> np.ndarray:
    meta = _preprocess(np.asarray(inputs["edge_index"]))
    nc = _build_program(meta)
    in_maps = _host_inputs(inputs, meta)
    res = run_bass_kernel_spmd(nc, in_maps, list(range(NCORES)))
    out = np.empty((N, OUT), np.float32)
    for c in range(NCORES):
        out[c * NPC_REAL:(c + 1) * NPC_REAL] = \
            res.results[c]["outT"][:, :NPC_REAL].T
    return out


# revision 19
# speedup vs baseline: 1.3674x; 1.1844x over previous
"""Two-layer GCN (PyG GCNConv x2 + ReLU) on 8 Trainium2 NeuronCores.

Strategy (graph/data parallel, per the destination-partitioned sharding):
  - Nodes are row-sharded across 8 cores (6250 real + pad -> 6272 per core).
  - Edges (plus one self-edge per node, which realises the GCN self-loop
    term exactly) are partitioned by destination owner and grouped by
    destination tile (128 dst nodes), then by source-row region
    (lo: table row < 32768, hi: >= 32768) so gather indices fit in int16.
  - Per layer on each core:  h^T = W^T @ x^T on PE;  g^T = h^T * dinv
    (the symmetric norm dinv[src]*dinv[dst] folds into per-node scaling);
    g rows are written to DRAM and AllGather'ed into a replicated node
    table;  per-edge source rows are fetched with SWDGE dma_gather;  the
    segment-sum by destination is a PE matmul against a one-hot S matrix
    built on the vector engine (iota == dstid);  epilogue scales by
    dinv[dst], adds bias and applies ReLU.
  - fp16 operands with fp32 PSUM accumulation.
"""

import math
import os
import sys

import numpy as np

for _p in ("/opt/trn_rl_repo", "/root/.axon_site/_ro/trn_rl_repo"):
    if os.path.isdir(_p) and _p not in sys.path:
        sys.path.append(_p)

import concourse.bacc as bacc
import concourse.bass as bass
import concourse.mybir as mybir
import concourse.tile as tile
from concourse.bass_utils import run_bass_kernel_spmd

# Problem constants (hardcoded per harness contract).
N, E, IN, HID, OUT = 50000, 800000, 128, 128, 64
NCORES = 8
NPC_REAL = N // NCORES          # 6250
TILES = 49
NPC = TILES * 128               # 6272 padded nodes per core
R = NCORES * NPC                # 50176 table rows
LO = 32768                      # int16-reachable row count
WV = 32                         # gather wave size in chunks (128 slots each)


def default_cfg():
    return dict(N=N, E=E, IN=IN, HID=HID, OUT=OUT, NCORES=NCORES,
                NPC_REAL=NPC_REAL, TILES=TILES, NPC=NPC, R=R, LO=LO, WV=WV)

F16 = mybir.dt.float16
F32 = mybir.dt.float32
NPF16 = np.float16

_ts = bass.ts


def _preprocess(edge_index: np.ndarray, cfg=None):
    """Partition/sort/pad edges; build per-core gather-index and dst-id
    arrays plus the shared static chunk schedule."""
    g = cfg or default_cfg()
    N, NCORES, NPC_REAL, TILES, NPC, LO = (
        g["N"], g["NCORES"], g["NPC_REAL"], g["TILES"], g["NPC"], g["LO"])
    src = np.asarray(edge_index[0], np.int64)
    dst = np.asarray(edge_index[1], np.int64)
    deg = np.bincount(dst, minlength=N).astype(np.float64) + 1.0

    selfn = np.arange(N, dtype=np.int64)
    src_all = np.concatenate([src, selfn])
    dst_all = np.concatenate([dst, selfn])

    owner = dst_all // NPC_REAL
    dst_local = dst_all % NPC_REAL
    src_row = (src_all // NPC_REAL) * NPC + (src_all % NPC_REAL)
    tile_id = dst_local // 128
    intile = dst_local % 128
    region = (src_row >= LO).astype(np.int64)

    cnt = np.zeros((NCORES, TILES, 2), np.int64)
    np.add.at(cnt, (owner, tile_id, region), 1)
    K = np.ceil(cnt.max(axis=0) / 128).astype(np.int64)   # [TILES, 2]
    K_lo, K_hi = K[:, 0].copy(), K[:, 1].copy()
    C_lo, C_hi = int(K_lo.sum()), int(K_hi.sum())
    C = C_lo + C_hi
    LB = np.concatenate([[0], np.cumsum(K_lo)[:-1]]).astype(np.int64)
    HB = (C_lo + np.concatenate([[0], np.cumsum(K_hi)[:-1]])).astype(np.int64)

    # slot assignment: sort edges by (owner, region, tile); position within
    # each (owner, tile, region) group via cumulative count
    gid = (owner * TILES + tile_id) * 2 + region
    order = np.lexsort((src_row, gid))
    gs = gid[order]
    starts = np.concatenate([[0], np.flatnonzero(np.diff(gs)) + 1])
    group_of = np.searchsorted(starts, np.arange(len(gs)), side="right") - 1
    pos = np.arange(len(gs)) - starts[group_of]

    base_chunk = np.where(region == 0, LB[tile_id], HB[tile_id])
    slot = np.empty(len(gs), np.int64)
    slot[order] = base_chunk[order] * 128 + pos

    nslots = C * 128
    idx16 = np.zeros((NCORES, nslots), np.int16)
    dstid = np.full((NCORES, nslots), -1.0, np.float32)
    idx16[owner, slot] = (src_row - region * LO).astype(np.int16)
    dstid[owner, slot] = intile

    # wrapped+replicated gather index tile [128, C*8]
    idx_t = idx16.reshape(NCORES, C * 8, 16).transpose(0, 2, 1)     # [8,16,C*8]
    idx_t = np.tile(idx_t, (1, 8, 1)).copy()                        # [8,128,C*8]
    # host-built one-hot S: [NCORES, 128(slot-in-chunk), C*128(chunk,dstcol)]
    ds = dstid.reshape(NCORES, C, 128)                  # [8, C, 128slot]
    sall = (ds[:, :, :, None] == np.arange(128, dtype=np.float32)[None, None, None, :])
    sall = sall.astype(NPF16).transpose(0, 2, 1, 3).reshape(NCORES, 128, C * 128).copy()

    return dict(deg=deg, K_lo=K_lo, K_hi=K_hi, C_lo=C_lo, C_hi=C_hi, C=C,
                LB=LB, HB=HB, idx_t=idx_t, sall=sall)


def _waves(n_chunks: int, chunk0: int, wv: int = WV):
    out = []
    c = 0
    while c < n_chunks:
        n = min(wv, n_chunks - c)
        out.append((chunk0 + c, n))
        c += n
    return out


def _build_program(meta, cfg=None):
    g = cfg or default_cfg()
    IN, HID, OUT = g["IN"], g["HID"], g["OUT"]
    NCORES, TILES, NPC, R, LO, WV = (g["NCORES"], g["TILES"], g["NPC"],
                                     g["R"], g["LO"], g["WV"])
    stage = g.get("stage", "full")   # "ag" | "gather" | "full"
    K_lo, K_hi = meta["K_lo"], meta["K_hi"]
    C_lo, C_hi, C = meta["C_lo"], meta["C_hi"], meta["C"]
    LB, HB = meta["LB"], meta["HB"]

    nc = bacc.Bacc("TRN2", target_bir_lowering=False, debug=False,
                   num_devices=NCORES, num_swdge_queues=4)

    # ---- I/O ----
    xT_d = nc.dram_tensor("xT", [IN, NPC], F16, kind="ExternalInput")
    w1_d = nc.dram_tensor("W1", [IN, HID], F16, kind="ExternalInput")
    w2_d = nc.dram_tensor("W2", [HID, OUT], F16, kind="ExternalInput")
    b1_d = nc.dram_tensor("b1c", [HID, 1], F32, kind="ExternalInput")
    b2_d = nc.dram_tensor("b2c", [OUT, 1], F32, kind="ExternalInput")
    deg_d = nc.dram_tensor("degrep", [128, NPC], F32, kind="ExternalInput")
    ident_d = nc.dram_tensor("ident", [128, 128], F16, kind="ExternalInput")
    idx_d = nc.dram_tensor("idxt", [128, C * 8], mybir.dt.int16,
                           kind="ExternalInput")
    s_d = nc.dram_tensor("sall", [128, C * 128], F16, kind="ExternalInput")
    out_d = nc.dram_tensor("outT", [OUT, NPC], F32, kind="ExternalOutput")

    # ---- internal DRAM (collective bounce + replicated tables) ----
    gdram = [nc.dram_tensor(f"gdram{l}", [NPC, 128], F16) for l in (1, 2)]
    table = [nc.dram_tensor(f"table{l}", [R, 128], F16, addr_space="Shared")
             for l in (1, 2)]

    rg = [list(range(NCORES))]

    with tile.TileContext(nc) as tc:
        with (
            tc.tile_pool(name="const", bufs=1) as constp,
            tc.tile_pool(name="big", bufs=2) as bigp,
            tc.tile_pool(name="big32", bufs=1) as big32p,
            tc.tile_pool(name="glo", bufs=2) as glop,
            tc.tile_pool(name="ghi", bufs=2) as ghip,
            tc.tile_pool(name="slo", bufs=2) as slop,
            tc.tile_pool(name="shi", bufs=2) as ship,
            tc.tile_pool(name="tmp", bufs=4) as tmpp,
            tc.tile_pool(name="pmm", bufs=2, space="PSUM") as pmm,
            tc.tile_pool(name="ptr", bufs=2, space="PSUM") as ptr,
            tc.tile_pool(name="psc", bufs=4, space="PSUM") as psc,
        ):
            # ---- constants / inputs to SBUF ----
            xT = bigp.tile([128, NPC], F16, tag="bigf16")
            nc.sync.dma_start(xT[:IN, :], xT_d[:, :])
            w1 = constp.tile([IN, HID], F16, tag="w1")
            nc.sync.dma_start(w1[:], w1_d[:, :])
            w2 = constp.tile([HID, OUT], F16, tag="w2")
            nc.sync.dma_start(w2[:], w2_d[:, :])
            b1 = constp.tile([HID, 1], F32, tag="b1")
            nc.sync.dma_start(b1[:], b1_d[:, :])
            b2 = constp.tile([OUT, 1], F32, tag="b2")
            nc.sync.dma_start(b2[:], b2_d[:, :])
            ident = constp.tile([128, 128], F16, tag="ident")
            nc.sync.dma_start(ident[:], ident_d[:, :])
            idxt = constp.tile([128, C * 8], mybir.dt.int16, tag="idxt")
            nc.sync.dma_start(idxt[:], idx_d[:, :])

            # dinv_rep = sqrt(1/deg), partition-replicated, f16 in SBUF
            degt = big32p.tile([128, NPC], F32, tag="bigf32")
            nc.sync.dma_start(degt[:], deg_d[:, :])
            nc.vector.reciprocal(degt[:], degt[:])
            dinv = constp.tile([128, NPC], F16, tag="dinv")
            nc.scalar.sqrt(dinv[:], degt[:])

            relu1 = None

            for layer in (0, 1):
                Fdim = HID if layer == 0 else OUT
                W = w1 if layer == 0 else w2
                bcol = b1 if layer == 0 else b2
                rhs_in = xT if layer == 0 else relu1

                # ---- 1. h^T = W^T @ rhs ; g^T = h^T * dinv (fp16) ----
                gT = bigp.tile([128, NPC], F16, tag="bigf16")
                nmm = math.ceil(NPC / 512)
                for i in range(nmm):
                    w_ = min(512, NPC - i * 512)
                    sl = slice(i * 512, i * 512 + w_)
                    ps = pmm.tile([128, 512], F32, tag="pmm")
                    nc.tensor.matmul(ps[:Fdim, :w_], W[:, :Fdim],
                                     rhs_in[:128, sl],
                                     start=True, stop=True)
                    nc.vector.scalar_tensor_tensor(
                        gT[:Fdim, sl], ps[:Fdim, :w_], 0.0,
                        dinv[:Fdim, sl],
                        mybir.AluOpType.bypass, mybir.AluOpType.mult)

                # ---- 2. transpose per dst tile into row-major staged ----
                staged = bigp.tile([128, NPC], F16, tag="bigf16")
                if Fdim < 128:
                    nc.vector.memset(staged[:], 0.0)
                for t in range(TILES):
                    pt = ptr.tile([128, Fdim], F16, tag="ptr")
                    nc.tensor.transpose(pt[:, :], gT[:Fdim, _ts(t, 128)],
                                        ident[:Fdim, :Fdim])
                    nc.vector.tensor_copy(staged[:, _ts(t, 128)][:, :Fdim],
                                          pt[:, :])

                # ---- 3. staged -> DRAM rows; AllGather into table ----
                gview = gdram[layer].ap().rearrange("(t p) f -> p t f", p=128)
                sview = staged[:].rearrange("p (t f) -> p t f", f=128)
                nc.sync.dma_start(gview, sview)
                nc.gpsimd.collective_compute(
                    "AllGather", mybir.AluOpType.bypass, replica_groups=rg,
                    ins=[gdram[layer].ap()], outs=[table[layer].ap()])

                # ---- 4. gather waves + one-hot scatter matmuls ----
                lo_waves = _waves(C_lo, 0, WV)
                hi_waves = _waves(C_hi, C_lo, WV)
                wave_tiles = {}

                def ensure_wave(rgn, wi, _wt=wave_tiles, _lw=lo_waves,
                                _hw=hi_waves, _ly=layer):
                    key = (rgn, wi)
                    if key in _wt:
                        return _wt[key]
                    c0, n = (_lw if rgn == 0 else _hw)[wi]
                    pool = glop if rgn == 0 else ghip
                    g = pool.tile([128, WV, 128], F16,
                                  tag="glo" if rgn == 0 else "ghi")
                    src = (table[_ly][0:LO, :] if rgn == 0
                           else table[_ly][LO:R, :])
                    qn = ensure_wave.q[0]
                    ensure_wave.q[0] = (qn + 1) % 4
                    nc.gpsimd.dma_gather(
                        g[:, :n, :], src, idxt[:, c0 * 8:(c0 + n) * 8],
                        n * 128, n * 128, 128, single_packet=False,
                        queue_num=qn)
                    _wt[key] = g
                    return g

                ensure_wave.q = [0]
                s_tiles = {}

                def ensure_s(rgn, wi, _st=s_tiles, _lw=lo_waves, _hw=hi_waves):
                    key = (rgn, wi)
                    if key in _st:
                        return _st[key]
                    c0, n = (_lw if rgn == 0 else _hw)[wi]
                    pool = slop if rgn == 0 else ship
                    st = pool.tile([128, WV * 128], F16,
                                   tag="slo" if rgn == 0 else "shi")
                    nc.sync.dma_start(st[:, :n * 128],
                                      s_d[:, c0 * 128:(c0 + n) * 128])
                    _st[key] = st
                    return st

                target = None
                if layer == 0:
                    relu1 = bigp.tile([128, NPC], F16, tag="bigf16")
                    target = relu1
                else:
                    outs = big32p.tile([OUT, NPC], F32, tag="bigf32")
                    target = outs

                if stage == "ag":
                    nc.vector.memset(target[:Fdim, :], 0.0)
                    continue
                if stage == "gather":
                    for wi in range(len(lo_waves)):
                        ensure_wave(0, wi)
                    for wi in range(len(hi_waves)):
                        ensure_wave(1, wi)
                    nc.vector.memset(target[:Fdim, :], 0.0)
                    continue

                for t in range(TILES):
                    nchunks = int(K_lo[t] + K_hi[t])
                    if nchunks == 0:
                        nc.vector.memset(target[:Fdim, _ts(t, 128)], 0.0)
                        continue
                    pscat = psc.tile([Fdim, 128], F32, tag="psc")
                    ci = 0
                    for rgn, Kr, Bs, off in ((0, K_lo, LB, 0),
                                             (1, K_hi, HB, C_lo)):
                        for k in range(int(Kr[t])):
                            ch = int(Bs[t]) + k          # global chunk id
                            rel = ch - off               # chunk id in region
                            g = ensure_wave(rgn, rel // WV)
                            pos = rel % WV
                            sw = ensure_s(rgn, rel // WV)
                            nc.tensor.matmul(
                                pscat[:Fdim, :], g[:, pos, :Fdim],
                                sw[:, pos * 128:(pos + 1) * 128],
                                start=(ci == 0), stop=(ci == nchunks - 1))
                            ci += 1

                    # ---- 5. epilogue: *dinv[dst], +bias, ReLU ----
                    if stage == "noepi":
                        nc.vector.memset(target[:Fdim, _ts(t, 128)], 0.0)
                        continue
                    tmp = tmpp.tile([Fdim, 128], F32, tag="tmp")
                    nc.vector.scalar_tensor_tensor(
                        tmp[:Fdim, :], pscat[:Fdim, :], 0.0,
                        dinv[:Fdim, _ts(t, 128)],
                        mybir.AluOpType.bypass, mybir.AluOpType.mult)
                    nc.vector.tensor_scalar(
                        target[:Fdim, _ts(t, 128)], tmp[:Fdim, :],
                        bcol[:Fdim, :], 0.0,
                        mybir.AluOpType.add, mybir.AluOpType.max)

            nc.sync.dma_start(out_d[:, :], target[:OUT, :])

    nc.compile()
    return nc


def _host_inputs(inputs, meta, cfg=None):
    g = cfg or default_cfg()
    N, IN, HID, OUT = g["N"], g["IN"], g["HID"], g["OUT"]
    NCORES, NPC_REAL, NPC = g["NCORES"], g["NPC_REAL"], g["NPC"]
    x = np.asarray(inputs["x"], np.float32)
    W1 = np.asarray(inputs["W1"], np.float32)
    b1 = np.asarray(inputs["b1"], np.float32)
    W2 = np.asarray(inputs["W2"], np.float32)
    b2 = np.asarray(inputs["b2"], np.float32)
    deg = meta["deg"]

    ident = np.eye(128, dtype=NPF16)
    w1c = W1.astype(NPF16)
    w2c = np.zeros((HID, OUT), NPF16)
    w2c[:, :] = W2.astype(NPF16)
    b1c = b1.reshape(HID, 1).astype(np.float32)
    b2c = b2.reshape(OUT, 1).astype(np.float32)

    in_maps = []
    for c in range(NCORES):
        xs = np.zeros((NPC, IN), np.float32)
        xs[:NPC_REAL] = x[c * NPC_REAL:(c + 1) * NPC_REAL]
        xT = np.ascontiguousarray(xs.T).astype(NPF16)

        node = np.arange(NPC) + c * NPC_REAL
        degs = np.ones(NPC, np.float32)
        degs[:NPC_REAL] = deg[node[:NPC_REAL]]
        degrep = np.tile(degs[None, :], (128, 1)).astype(np.float32)

        in_maps.append({
            "xT": xT, "W1": w1c, "W2": w2c, "b1c": b1c, "b2c": b2c,
            "degrep": degrep, "ident": ident,
            "idxt": meta["idx_t"][c], "sall": meta["sall"][c],
        })
    return in_maps


def kernel(**inputs) -> np.ndarray:
    meta = _preprocess(np.asarray(inputs["edge_index"]))
    nc = _build_program(meta)
    in_maps = _host_inputs(inputs, meta)
    res = run_bass_kernel_spmd(nc, in_maps, list(range(NCORES)))
    out = np.empty((N, OUT), np.float32)
    for c in range(NCORES):
        out[c * NPC_REAL:(c + 1) * NPC_REAL] = \
            res.results[c]["outT"][:, :NPC_REAL].T
    return out


# revision 20
# speedup vs baseline: 1.4123x; 1.0328x over previous
"""Two-layer GCN (PyG GCNConv x2 + ReLU) on 8 Trainium2 NeuronCores.

Strategy (graph/data parallel, per the destination-partitioned sharding):
  - Nodes are row-sharded across 8 cores (6250 real + pad -> 6272 per core).
  - Edges (plus one self-edge per node, which realises the GCN self-loop
    term exactly) are partitioned by destination owner and grouped by
    destination tile (128 dst nodes), then by source-row region
    (lo: table row < 32768, hi: >= 32768) so gather indices fit in int16.
  - Per layer on each core:  h^T = W^T @ x^T on PE;  g^T = h^T * dinv
    (the symmetric norm dinv[src]*dinv[dst] folds into per-node scaling);
    g rows are written to DRAM and AllGather'ed into a replicated node
    table;  per-edge source rows are fetched with SWDGE dma_gather;  the
    segment-sum by destination is a PE matmul against a one-hot S matrix
    built on the vector engine (iota == dstid);  epilogue scales by
    dinv[dst], adds bias and applies ReLU.
  - fp16 operands with fp32 PSUM accumulation.
"""

import math
import os
import sys

import numpy as np

for _p in ("/opt/trn_rl_repo", "/root/.axon_site/_ro/trn_rl_repo"):
    if os.path.isdir(_p) and _p not in sys.path:
        sys.path.append(_p)

import concourse.bacc as bacc
import concourse.bass as bass
import concourse.mybir as mybir
import concourse.tile as tile
from concourse.bass_utils import run_bass_kernel_spmd

# Problem constants (hardcoded per harness contract).
N, E, IN, HID, OUT = 50000, 800000, 128, 128, 64
NCORES = 8
NPC_REAL = N // NCORES          # 6250
TILES = 49
NPC = TILES * 128               # 6272 padded nodes per core
R = NCORES * NPC                # 50176 table rows
LO = 32768                      # int16-reachable row count
WV = 32                         # gather wave size in chunks (128 slots each)


def default_cfg():
    return dict(N=N, E=E, IN=IN, HID=HID, OUT=OUT, NCORES=NCORES,
                NPC_REAL=NPC_REAL, TILES=TILES, NPC=NPC, R=R, LO=LO, WV=WV)

F16 = mybir.dt.float16
F32 = mybir.dt.float32
NPF16 = np.float16

_ts = bass.ts


def _preprocess(edge_index: np.ndarray, cfg=None):
    """Partition/sort/pad edges; build per-core gather-index and dst-id
    arrays plus the shared static chunk schedule."""
    g = cfg or default_cfg()
    N, NCORES, NPC_REAL, TILES, NPC, LO = (
        g["N"], g["NCORES"], g["NPC_REAL"], g["TILES"], g["NPC"], g["LO"])
    src = np.asarray(edge_index[0], np.int64)
    dst = np.asarray(edge_index[1], np.int64)
    deg = np.bincount(dst, minlength=N).astype(np.float64) + 1.0

    selfn = np.arange(N, dtype=np.int64)
    src_all = np.concatenate([src, selfn])
    dst_all = np.concatenate([dst, selfn])

    owner = dst_all // NPC_REAL
    dst_local = dst_all % NPC_REAL
    src_row = (src_all // NPC_REAL) * NPC + (src_all % NPC_REAL)
    tile_id = dst_local // 128
    intile = dst_local % 128
    region = (src_row >= LO).astype(np.int64)

    cnt = np.zeros((NCORES, TILES, 2), np.int64)
    np.add.at(cnt, (owner, tile_id, region), 1)
    K = np.ceil(cnt.max(axis=0) / 128).astype(np.int64)   # [TILES, 2]
    K_lo, K_hi = K[:, 0].copy(), K[:, 1].copy()
    C_lo, C_hi = int(K_lo.sum()), int(K_hi.sum())
    C = C_lo + C_hi
    LB = np.concatenate([[0], np.cumsum(K_lo)[:-1]]).astype(np.int64)
    HB = (C_lo + np.concatenate([[0], np.cumsum(K_hi)[:-1]])).astype(np.int64)

    # slot assignment: sort edges by (owner, region, tile); position within
    # each (owner, tile, region) group via cumulative count
    gid = (owner * TILES + tile_id) * 2 + region
    order = np.lexsort((src_row, gid))
    gs = gid[order]
    starts = np.concatenate([[0], np.flatnonzero(np.diff(gs)) + 1])
    group_of = np.searchsorted(starts, np.arange(len(gs)), side="right") - 1
    pos = np.arange(len(gs)) - starts[group_of]

    base_chunk = np.where(region == 0, LB[tile_id], HB[tile_id])
    slot = np.empty(len(gs), np.int64)
    slot[order] = base_chunk[order] * 128 + pos

    nslots = C * 128
    idx16 = np.zeros((NCORES, nslots), np.int16)
    dstid = np.full((NCORES, nslots), -1.0, np.float32)
    idx16[owner, slot] = (src_row - region * LO).astype(np.int16)
    dstid[owner, slot] = intile

    # wrapped+replicated gather index tile [128, C*8]
    idx_t = idx16.reshape(NCORES, C * 8, 16).transpose(0, 2, 1)     # [8,16,C*8]
    idx_t = np.tile(idx_t, (1, 8, 1)).copy()                        # [8,128,C*8]
    # host-built one-hot S: [NCORES, 128(slot-in-chunk), C*128(chunk,dstcol)]
    ds = dstid.reshape(NCORES, C, 128)                  # [8, C, 128slot]
    sall = (ds[:, :, :, None] == np.arange(128, dtype=np.float32)[None, None, None, :])
    sall = sall.astype(NPF16).transpose(0, 2, 1, 3).reshape(NCORES, 128, C * 128).copy()

    return dict(deg=deg, K_lo=K_lo, K_hi=K_hi, C_lo=C_lo, C_hi=C_hi, C=C,
                LB=LB, HB=HB, idx_t=idx_t, sall=sall)


def _waves(n_chunks: int, chunk0: int, wv: int = WV):
    out = []
    c = 0
    while c < n_chunks:
        n = min(wv, n_chunks - c)
        out.append((chunk0 + c, n))
        c += n
    return out


def _build_program(meta, cfg=None):
    g = cfg or default_cfg()
    IN, HID, OUT = g["IN"], g["HID"], g["OUT"]
    NCORES, TILES, NPC, R, LO, WV = (g["NCORES"], g["TILES"], g["NPC"],
                                     g["R"], g["LO"], g["WV"])
    stage = g.get("stage", "full")   # "ag" | "gather" | "full"
    K_lo, K_hi = meta["K_lo"], meta["K_hi"]
    C_lo, C_hi, C = meta["C_lo"], meta["C_hi"], meta["C"]
    LB, HB = meta["LB"], meta["HB"]

    nc = bacc.Bacc("TRN2", target_bir_lowering=False, debug=False,
                   num_devices=NCORES, num_swdge_queues=4)

    # ---- I/O ----
    xT_d = nc.dram_tensor("xT", [IN, NPC], F16, kind="ExternalInput")
    w1_d = nc.dram_tensor("W1", [IN, HID], F16, kind="ExternalInput")
    w2_d = nc.dram_tensor("W2", [HID, OUT], F16, kind="ExternalInput")
    b1_d = nc.dram_tensor("b1c", [HID, 1], F32, kind="ExternalInput")
    b2_d = nc.dram_tensor("b2c", [OUT, 1], F32, kind="ExternalInput")
    deg_d = nc.dram_tensor("degrep", [128, NPC], F32, kind="ExternalInput")
    ident_d = nc.dram_tensor("ident", [128, 128], F16, kind="ExternalInput")
    idx_d = nc.dram_tensor("idxt", [128, C * 8], mybir.dt.int16,
                           kind="ExternalInput")
    s_d = nc.dram_tensor("sall", [128, C * 128], F16, kind="ExternalInput")
    out_d = nc.dram_tensor("outT", [OUT, NPC], F32, kind="ExternalOutput")

    # ---- internal DRAM (collective bounce + replicated tables) ----
    gdram = [nc.dram_tensor(f"gdram{l}", [NPC, 128], F16) for l in (1, 2)]
    table = [nc.dram_tensor(f"table{l}", [R, 128], F16, addr_space="Shared")
             for l in (1, 2)]

    rg = [list(range(NCORES))]

    with tile.TileContext(nc) as tc:
        with (
            tc.tile_pool(name="const", bufs=1) as constp,
            tc.tile_pool(name="big", bufs=2) as bigp,
            tc.tile_pool(name="big32", bufs=1) as big32p,
            tc.tile_pool(name="glo", bufs=2) as glop,
            tc.tile_pool(name="ghi", bufs=2) as ghip,
            tc.tile_pool(name="slo", bufs=2) as slop,
            tc.tile_pool(name="shi", bufs=2) as ship,
            tc.tile_pool(name="tmp", bufs=4) as tmpp,
            tc.tile_pool(name="pmm", bufs=2, space="PSUM") as pmm,
            tc.tile_pool(name="ptr", bufs=2, space="PSUM") as ptr,
            tc.tile_pool(name="psc", bufs=4, space="PSUM") as psc,
        ):
            # ---- constants / inputs to SBUF ----
            xT = bigp.tile([128, NPC], F16, tag="bigf16")
            nc.sync.dma_start(xT[:IN, :], xT_d[:, :])
            w1 = constp.tile([IN, HID], F16, tag="w1")
            nc.sync.dma_start(w1[:], w1_d[:, :])
            w2 = constp.tile([HID, OUT], F16, tag="w2")
            nc.sync.dma_start(w2[:], w2_d[:, :])
            b1 = constp.tile([HID, 1], F32, tag="b1")
            nc.sync.dma_start(b1[:], b1_d[:, :])
            b2 = constp.tile([OUT, 1], F32, tag="b2")
            nc.sync.dma_start(b2[:], b2_d[:, :])
            ident = constp.tile([128, 128], F16, tag="ident")
            nc.sync.dma_start(ident[:], ident_d[:, :])
            idxt = constp.tile([128, C * 8], mybir.dt.int16, tag="idxt")
            nc.sync.dma_start(idxt[:], idx_d[:, :])

            # dinv_rep = sqrt(1/deg), partition-replicated, f16 in SBUF
            degt = big32p.tile([128, NPC], F32, tag="bigf32")
            nc.sync.dma_start(degt[:], deg_d[:, :])
            nc.vector.reciprocal(degt[:], degt[:])
            dinv = constp.tile([128, NPC], F16, tag="dinv")
            nc.scalar.sqrt(dinv[:], degt[:])

            relu1 = None

            for layer in (0, 1):
                Fdim = HID if layer == 0 else OUT
                W = w1 if layer == 0 else w2
                bcol = b1 if layer == 0 else b2
                rhs_in = xT if layer == 0 else relu1

                # ---- 1. h^T = W^T @ rhs ; g^T = h^T * dinv (fp16) ----
                gT = bigp.tile([128, NPC], F16, tag="bigf16")
                nmm = math.ceil(NPC / 512)
                for i in range(nmm):
                    w_ = min(512, NPC - i * 512)
                    sl = slice(i * 512, i * 512 + w_)
                    ps = pmm.tile([128, 512], F32, tag="pmm")
                    nc.tensor.matmul(ps[:Fdim, :w_], W[:, :Fdim],
                                     rhs_in[:128, sl],
                                     start=True, stop=True)
                    nc.vector.scalar_tensor_tensor(
                        gT[:Fdim, sl], ps[:Fdim, :w_], 0.0,
                        dinv[:Fdim, sl],
                        mybir.AluOpType.bypass, mybir.AluOpType.mult)

                # ---- 2. transpose per dst tile into row-major staged ----
                staged = bigp.tile([128, NPC], F16, tag="bigf16")
                if Fdim < 128:
                    nc.vector.memset(staged[:], 0.0)
                for t in range(TILES):
                    pt = ptr.tile([128, Fdim], F16, tag="ptr")
                    nc.tensor.transpose(pt[:, :], gT[:Fdim, _ts(t, 128)],
                                        ident[:Fdim, :Fdim])
                    nc.vector.tensor_copy(staged[:, _ts(t, 128)][:, :Fdim],
                                          pt[:, :])

                # ---- 3. staged -> DRAM rows; AllGather into table ----
                gview = gdram[layer].ap().rearrange("(t p) f -> p t f", p=128)
                sview = staged[:].rearrange("p (t f) -> p t f", f=128)
                nc.sync.dma_start(gview, sview)
                nc.gpsimd.collective_compute(
                    "AllGather", mybir.AluOpType.bypass, replica_groups=rg,
                    ins=[gdram[layer].ap()], outs=[table[layer].ap()])

                # ---- 4. gather waves + one-hot scatter matmuls ----
                lo_waves = _waves(C_lo, 0, WV)
                hi_waves = _waves(C_hi, C_lo, WV)
                wave_tiles = {}

                def ensure_wave(rgn, wi, _wt=wave_tiles, _lw=lo_waves,
                                _hw=hi_waves, _ly=layer):
                    key = (rgn, wi)
                    if key in _wt:
                        return _wt[key]
                    c0, n = (_lw if rgn == 0 else _hw)[wi]
                    pool = glop if rgn == 0 else ghip
                    g = pool.tile([128, WV, 128], F16,
                                  tag="glo" if rgn == 0 else "ghi")
                    src = (table[_ly][0:LO, :] if rgn == 0
                           else table[_ly][LO:R, :])
                    qn = ensure_wave.q[0]
                    ensure_wave.q[0] = (qn + 1) % 4
                    nc.gpsimd.dma_gather(
                        g[:, :n, :], src, idxt[:, c0 * 8:(c0 + n) * 8],
                        n * 128, n * 128, 128, single_packet=False,
                        queue_num=qn)
                    _wt[key] = g
                    return g

                ensure_wave.q = [0]
                s_tiles = {}

                def ensure_s(rgn, wi, _st=s_tiles, _lw=lo_waves, _hw=hi_waves):
                    key = (rgn, wi)
                    if key in _st:
                        return _st[key]
                    c0, n = (_lw if rgn == 0 else _hw)[wi]
                    pool = slop if rgn == 0 else ship
                    st = pool.tile([128, WV * 128], F16,
                                   tag="slo" if rgn == 0 else "shi")
                    nc.sync.dma_start(st[:, :n * 128],
                                      s_d[:, c0 * 128:(c0 + n) * 128])
                    _st[key] = st
                    return st

                target = None
                if layer == 0:
                    relu1 = bigp.tile([128, NPC], F16, tag="bigf16")
                    target = relu1
                else:
                    outs = big32p.tile([OUT, NPC], F32, tag="bigf32")
                    target = outs

                if stage == "ag":
                    nc.vector.memset(target[:Fdim, :], 0.0)
                    continue
                if stage == "gather":
                    for wi in range(len(lo_waves)):
                        ensure_wave(0, wi)
                    for wi in range(len(hi_waves)):
                        ensure_wave(1, wi)
                    nc.vector.memset(target[:Fdim, :], 0.0)
                    continue

                for t in range(TILES):
                    nchunks = int(K_lo[t] + K_hi[t])
                    if nchunks == 0:
                        nc.vector.memset(target[:Fdim, _ts(t, 128)], 0.0)
                        continue
                    pscat = psc.tile([Fdim, 128], F32, tag="psc")
                    ci = 0
                    for rgn, Kr, Bs, off in ((0, K_lo, LB, 0),
                                             (1, K_hi, HB, C_lo)):
                        for k in range(int(Kr[t])):
                            ch = int(Bs[t]) + k          # global chunk id
                            rel = ch - off               # chunk id in region
                            g = ensure_wave(rgn, rel // WV)
                            pos = rel % WV
                            sw = ensure_s(rgn, rel // WV)
                            nc.tensor.matmul(
                                pscat[:Fdim, :], g[:, pos, :Fdim],
                                sw[:, pos * 128:(pos + 1) * 128],
                                start=(ci == 0), stop=(ci == nchunks - 1))
                            ci += 1

                    # ---- 5. epilogue: *dinv[dst], +bias, ReLU ----
                    if stage == "noepi":
                        nc.vector.memset(target[:Fdim, _ts(t, 128)], 0.0)
                        continue
                    tmp = tmpp.tile([Fdim, 128], F32, tag="tmp")
                    nc.vector.scalar_tensor_tensor(
                        tmp[:Fdim, :], pscat[:Fdim, :], 0.0,
                        dinv[:Fdim, _ts(t, 128)],
                        mybir.AluOpType.bypass, mybir.AluOpType.mult)
                    nc.scalar.activation(
                        target[:Fdim, _ts(t, 128)], tmp[:Fdim, :],
                        mybir.ActivationFunctionType.Relu,
                        bias=bcol[:Fdim, :], scale=1.0)

            nc.sync.dma_start(out_d[:, :], target[:OUT, :])

    nc.compile()
    return nc


def _host_inputs(inputs, meta, cfg=None):
    g = cfg or default_cfg()
    N, IN, HID, OUT = g["N"], g["IN"], g["HID"], g["OUT"]
    NCORES, NPC_REAL, NPC = g["NCORES"], g["NPC_REAL"], g["NPC"]
    x = np.asarray(inputs["x"], np.float32)
    W1 = np.asarray(inputs["W1"], np.float32)
    b1 = np.asarray(inputs["b1"], np.float32)
    W2 = np.asarray(inputs["W2"], np.float32)
    b2 = np.asarray(inputs["b2"], np.float32)
    deg = meta["deg"]

    ident = np.eye(128, dtype=NPF16)
    w1c = W1.astype(NPF16)
    w2c = np.zeros((HID, OUT), NPF16)
    w2c[:, :] = W2.astype(NPF16)
    b1c = b1.reshape(HID, 1).astype(np.float32)
    b2c = b2.reshape(OUT, 1).astype(np.float32)

    in_maps = []
    for c in range(NCORES):
        xs = np.zeros((NPC, IN), np.float32)
        xs[:NPC_REAL] = x[c * NPC_REAL:(c + 1) * NPC_REAL]
        xT = np.ascontiguousarray(xs.T).astype(NPF16)

        node = np.arange(NPC) + c * NPC_REAL
        degs = np.ones(NPC, np.float32)
        degs[:NPC_REAL] = deg[node[:NPC_REAL]]
        degrep = np.tile(degs[None, :], (128, 1)).astype(np.float32)

        in_maps.append({
            "xT": xT, "W1": w1c, "W2": w2c, "b1c": b1c, "b2c": b2c,
            "degrep": degrep, "ident": ident,
            "idxt": meta["idx_t"][c], "sall": meta["sall"][c],
        })
    return in_maps


def kernel(**inputs) -> np.ndarray:
    meta = _preprocess(np.asarray(inputs["edge_index"]))
    nc = _build_program(meta)
    in_maps = _host_inputs(inputs, meta)
    res = run_bass_kernel_spmd(nc, in_maps, list(range(NCORES)))
    out = np.empty((N, OUT), np.float32)
    for c in range(NCORES):
        out[c * NPC_REAL:(c + 1) * NPC_REAL] = \
            res.results[c]["outT"][:, :NPC_REAL].T
    return out


# revision 22
# speedup vs baseline: 1.4725x; 1.0426x over previous
"""Two-layer GCN (PyG GCNConv x2 + ReLU) on 8 Trainium2 NeuronCores.

Strategy (graph/data parallel, per the destination-partitioned sharding):
  - Nodes are row-sharded across 8 cores (6250 real + pad -> 6272 per core).
  - Edges (plus one self-edge per node, which realises the GCN self-loop
    term exactly) are partitioned by destination owner and grouped by
    destination tile (128 dst nodes), then by source-row region
    (lo: table row < 32768, hi: >= 32768) so gather indices fit in int16.
  - Per layer on each core:  h^T = W^T @ x^T on PE;  g^T = h^T * dinv
    (the symmetric norm dinv[src]*dinv[dst] folds into per-node scaling);
    g rows are written to DRAM and AllGather'ed into a replicated node
    table;  per-edge source rows are fetched with SWDGE dma_gather;  the
    segment-sum by destination is a PE matmul against a one-hot S matrix
    built on the vector engine (iota == dstid);  epilogue scales by
    dinv[dst], adds bias and applies ReLU.
  - fp16 operands with fp32 PSUM accumulation.
"""

import math
import os
import sys

import numpy as np

for _p in ("/opt/trn_rl_repo", "/root/.axon_site/_ro/trn_rl_repo"):
    if os.path.isdir(_p) and _p not in sys.path:
        sys.path.append(_p)

import concourse.bacc as bacc
import concourse.bass as bass
import concourse.mybir as mybir
import concourse.tile as tile
from concourse.bass_utils import run_bass_kernel_spmd

# Problem constants (hardcoded per harness contract).
N, E, IN, HID, OUT = 50000, 800000, 128, 128, 64
NCORES = 8
NPC_REAL = N // NCORES          # 6250
TILES = 49
NPC = TILES * 128               # 6272 padded nodes per core
R = NCORES * NPC                # 50176 table rows
LO = 32768                      # int16-reachable row count
WV = 32                         # gather wave size in chunks (128 slots each)


def default_cfg():
    return dict(N=N, E=E, IN=IN, HID=HID, OUT=OUT, NCORES=NCORES,
                NPC_REAL=NPC_REAL, TILES=TILES, NPC=NPC, R=R, LO=LO, WV=WV)

F16 = mybir.dt.float16
F32 = mybir.dt.float32
NPF16 = np.float16

_ts = bass.ts


def _preprocess(edge_index: np.ndarray, cfg=None):
    """Partition/sort/pad edges; build per-core gather-index and dst-id
    arrays plus the shared static chunk schedule."""
    g = cfg or default_cfg()
    N, NCORES, NPC_REAL, TILES, NPC, LO = (
        g["N"], g["NCORES"], g["NPC_REAL"], g["TILES"], g["NPC"], g["LO"])
    src = np.asarray(edge_index[0], np.int64)
    dst = np.asarray(edge_index[1], np.int64)
    deg = np.bincount(dst, minlength=N).astype(np.float64) + 1.0

    selfn = np.arange(N, dtype=np.int64)
    src_all = np.concatenate([src, selfn])
    dst_all = np.concatenate([dst, selfn])

    owner = dst_all // NPC_REAL
    dst_local = dst_all % NPC_REAL
    src_row = (src_all // NPC_REAL) * NPC + (src_all % NPC_REAL)
    tile_id = dst_local // 128
    intile = dst_local % 128
    region = (src_row >= LO).astype(np.int64)

    cnt = np.zeros((NCORES, TILES, 2), np.int64)
    np.add.at(cnt, (owner, tile_id, region), 1)
    K = np.ceil(cnt.max(axis=0) / 128).astype(np.int64)   # [TILES, 2]
    K_lo, K_hi = K[:, 0].copy(), K[:, 1].copy()
    C_lo, C_hi = int(K_lo.sum()), int(K_hi.sum())
    C = C_lo + C_hi
    LB = np.concatenate([[0], np.cumsum(K_lo)[:-1]]).astype(np.int64)
    HB = (C_lo + np.concatenate([[0], np.cumsum(K_hi)[:-1]])).astype(np.int64)

    # slot assignment: sort edges by (owner, region, tile); position within
    # each (owner, tile, region) group via cumulative count
    gid = (owner * TILES + tile_id) * 2 + region
    order = np.lexsort((src_row, gid))
    gs = gid[order]
    starts = np.concatenate([[0], np.flatnonzero(np.diff(gs)) + 1])
    group_of = np.searchsorted(starts, np.arange(len(gs)), side="right") - 1
    pos = np.arange(len(gs)) - starts[group_of]

    base_chunk = np.where(region == 0, LB[tile_id], HB[tile_id])
    slot = np.empty(len(gs), np.int64)
    slot[order] = base_chunk[order] * 128 + pos

    nslots = C * 128
    idx16 = np.zeros((NCORES, nslots), np.int16)
    dstid = np.full((NCORES, nslots), -1.0, np.float32)
    idx16[owner, slot] = (src_row - region * LO).astype(np.int16)
    dstid[owner, slot] = intile

    # wrapped+replicated gather index tile [128, C*8]
    idx_t = idx16.reshape(NCORES, C * 8, 16).transpose(0, 2, 1)     # [8,16,C*8]
    idx_t = np.tile(idx_t, (1, 8, 1)).copy()                        # [8,128,C*8]
    # host-built one-hot S: [NCORES, 128(slot-in-chunk), C*128(chunk,dstcol)]
    ds = dstid.reshape(NCORES, C, 128)                  # [8, C, 128slot]
    sall = (ds[:, :, :, None] == np.arange(128, dtype=np.float32)[None, None, None, :])
    sall = sall.astype(NPF16).transpose(0, 2, 1, 3).reshape(NCORES, 128, C * 128).copy()

    return dict(deg=deg, K_lo=K_lo, K_hi=K_hi, C_lo=C_lo, C_hi=C_hi, C=C,
                LB=LB, HB=HB, idx_t=idx_t, sall=sall)


def _waves(n_chunks: int, chunk0: int, wv: int = WV):
    out = []
    c = 0
    while c < n_chunks:
        n = min(wv, n_chunks - c)
        out.append((chunk0 + c, n))
        c += n
    return out


def _build_program(meta, cfg=None):
    g = cfg or default_cfg()
    IN, HID, OUT = g["IN"], g["HID"], g["OUT"]
    NCORES, TILES, NPC, R, LO, WV = (g["NCORES"], g["TILES"], g["NPC"],
                                     g["R"], g["LO"], g["WV"])
    stage = g.get("stage", "full")   # "ag" | "gather" | "full"
    K_lo, K_hi = meta["K_lo"], meta["K_hi"]
    C_lo, C_hi, C = meta["C_lo"], meta["C_hi"], meta["C"]
    LB, HB = meta["LB"], meta["HB"]

    nc = bacc.Bacc("TRN2", target_bir_lowering=False, debug=False,
                   num_devices=NCORES, num_swdge_queues=4)

    # ---- I/O ----
    xT_d = nc.dram_tensor("xT", [IN, NPC], F16, kind="ExternalInput")
    w1_d = nc.dram_tensor("W1", [IN, HID], F16, kind="ExternalInput")
    w2_d = nc.dram_tensor("W2", [HID, OUT], F16, kind="ExternalInput")
    b1_d = nc.dram_tensor("b1c", [HID, 1], F32, kind="ExternalInput")
    b2_d = nc.dram_tensor("b2c", [OUT, 1], F32, kind="ExternalInput")
    deg_d = nc.dram_tensor("degrep", [128, NPC], F32, kind="ExternalInput")
    ident_d = nc.dram_tensor("ident", [128, 128], F16, kind="ExternalInput")
    idx_d = nc.dram_tensor("idxt", [128, C * 8], mybir.dt.int16,
                           kind="ExternalInput")
    s_d = nc.dram_tensor("sall", [128, C * 128], F16, kind="ExternalInput")
    out_d = nc.dram_tensor("outT", [OUT, NPC], F32, kind="ExternalOutput")

    # ---- internal DRAM (collective bounce + replicated tables) ----
    gdram = [nc.dram_tensor(f"gdram{l}", [NPC, 128], F16) for l in (1, 2)]
    table = [nc.dram_tensor(f"table{l}", [R, 128], F16, addr_space="Shared")
             for l in (1, 2)]

    rg = [list(range(NCORES))]

    with tile.TileContext(nc) as tc:
        with (
            tc.tile_pool(name="const", bufs=1) as constp,
            tc.tile_pool(name="big", bufs=2) as bigp,
            tc.tile_pool(name="outp", bufs=3) as outp,
            tc.tile_pool(name="glo", bufs=4) as glop,
            tc.tile_pool(name="ghi", bufs=4) as ghip,
            tc.tile_pool(name="slo", bufs=2) as slop,
            tc.tile_pool(name="shi", bufs=2) as ship,
            tc.tile_pool(name="tmp", bufs=4) as tmpp,
            tc.tile_pool(name="pmm", bufs=2, space="PSUM") as pmm,
            tc.tile_pool(name="ptr", bufs=2, space="PSUM") as ptr,
            tc.tile_pool(name="psc", bufs=4, space="PSUM") as psc,
        ):
            # ---- constants / inputs to SBUF ----
            xT = bigp.tile([128, NPC], F16, tag="bigf16")
            nc.sync.dma_start(xT[:IN, :], xT_d[:, :])
            w1 = constp.tile([IN, HID], F16, tag="w1")
            nc.sync.dma_start(w1[:], w1_d[:, :])
            w2 = constp.tile([HID, OUT], F16, tag="w2")
            nc.sync.dma_start(w2[:], w2_d[:, :])
            b1 = constp.tile([HID, 1], F32, tag="b1")
            nc.sync.dma_start(b1[:], b1_d[:, :])
            b2 = constp.tile([OUT, 1], F32, tag="b2")
            nc.sync.dma_start(b2[:], b2_d[:, :])
            ident = constp.tile([128, 128], F16, tag="ident")
            nc.sync.dma_start(ident[:], ident_d[:, :])
            idxt = constp.tile([128, C * 8], mybir.dt.int16, tag="idxt")
            nc.sync.dma_start(idxt[:], idx_d[:, :])

            # dinv_rep = sqrt(1/deg), partition-replicated, f16 in SBUF
            dinv = constp.tile([128, NPC], F16, tag="dinv")
            for i in range(0, NPC, 1568):
                w_ = min(1568, NPC - i)
                degt = tmpp.tile([128, 1568], F32, tag="degt")
                nc.sync.dma_start(degt[:, :w_], deg_d[:, i:i + w_])
                nc.vector.reciprocal(degt[:, :w_], degt[:, :w_])
                nc.scalar.sqrt(dinv[:, i:i + w_], degt[:, :w_])

            relu1 = None

            for layer in (0, 1):
                Fdim = HID if layer == 0 else OUT
                W = w1 if layer == 0 else w2
                bcol = b1 if layer == 0 else b2
                rhs_in = xT if layer == 0 else relu1

                # ---- 1. h^T = W^T @ rhs ; g^T = h^T * dinv (fp16) ----
                gT = bigp.tile([128, NPC], F16, tag="bigf16")
                nmm = math.ceil(NPC / 512)
                for i in range(nmm):
                    w_ = min(512, NPC - i * 512)
                    sl = slice(i * 512, i * 512 + w_)
                    ps = pmm.tile([128, 512], F32, tag="pmm")
                    nc.tensor.matmul(ps[:Fdim, :w_], W[:, :Fdim],
                                     rhs_in[:128, sl],
                                     start=True, stop=True)
                    nc.vector.scalar_tensor_tensor(
                        gT[:Fdim, sl], ps[:Fdim, :w_], 0.0,
                        dinv[:Fdim, sl],
                        mybir.AluOpType.bypass, mybir.AluOpType.mult)

                # ---- 2. transpose per dst tile into row-major staged ----
                staged = bigp.tile([128, NPC], F16, tag="bigf16")
                if Fdim < 128:
                    nc.vector.memset(staged[:], 0.0)
                for t in range(TILES):
                    pt = ptr.tile([128, Fdim], F16, tag="ptr")
                    nc.tensor.transpose(pt[:, :], gT[:Fdim, _ts(t, 128)],
                                        ident[:Fdim, :Fdim])
                    nc.vector.tensor_copy(staged[:, _ts(t, 128)][:, :Fdim],
                                          pt[:, :])

                # ---- 3. staged -> DRAM rows; AllGather into table ----
                gview = gdram[layer].ap().rearrange("(t p) f -> p t f", p=128)
                sview = staged[:].rearrange("p (t f) -> p t f", f=128)
                nc.sync.dma_start(gview, sview)
                nc.gpsimd.collective_compute(
                    "AllGather", mybir.AluOpType.bypass, replica_groups=rg,
                    ins=[gdram[layer].ap()], outs=[table[layer].ap()])

                # ---- 4. gather waves + one-hot scatter matmuls ----
                lo_waves = _waves(C_lo, 0, WV)
                hi_waves = _waves(C_hi, C_lo, WV)
                WVS = 16
                slo_waves = _waves(C_lo, 0, WVS)
                shi_waves = _waves(C_hi, C_lo, WVS)
                wave_tiles = {}

                def ensure_wave(rgn, wi, _wt=wave_tiles, _lw=lo_waves,
                                _hw=hi_waves, _ly=layer):
                    key = (rgn, wi)
                    if key in _wt:
                        return _wt[key]
                    c0, n = (_lw if rgn == 0 else _hw)[wi]
                    pool = glop if rgn == 0 else ghip
                    g = pool.tile([128, WV, 128], F16,
                                  tag="glo" if rgn == 0 else "ghi")
                    src = (table[_ly][0:LO, :] if rgn == 0
                           else table[_ly][LO:R, :])
                    qn = ensure_wave.q[0]
                    ensure_wave.q[0] = (qn + 1) % 4
                    nc.gpsimd.dma_gather(
                        g[:, :n, :], src, idxt[:, c0 * 8:(c0 + n) * 8],
                        n * 128, n * 128, 128, single_packet=False,
                        queue_num=qn)
                    _wt[key] = g
                    return g

                ensure_wave.q = [0]
                s_tiles = {}

                def ensure_s(rgn, wi, _st=s_tiles, _lw=slo_waves,
                             _hw=shi_waves):
                    key = (rgn, wi)
                    if key in _st:
                        return _st[key]
                    c0, n = (_lw if rgn == 0 else _hw)[wi]
                    pool = slop if rgn == 0 else ship
                    st = pool.tile([128, WVS * 128], F16,
                                   tag="slo" if rgn == 0 else "shi")
                    nc.sync.dma_start(st[:, :n * 128],
                                      s_d[:, c0 * 128:(c0 + n) * 128])
                    _st[key] = st
                    return st

                target = None
                if layer == 0:
                    relu1 = bigp.tile([128, NPC], F16, tag="bigf16")
                    target = relu1

                if stage in ("ag", "gather"):
                    if stage == "gather":
                        for wi in range(len(lo_waves)):
                            ensure_wave(0, wi)
                        for wi in range(len(hi_waves)):
                            ensure_wave(1, wi)
                    if layer == 0:
                        nc.vector.memset(target[:, :], 0.0)
                    else:
                        for t in range(TILES):
                            ot = outp.tile([OUT, 128], F32, tag="out")
                            nc.vector.memset(ot[:], 0.0)
                            nc.sync.dma_start(out_d[:, _ts(t, 128)], ot[:])
                    continue

                for t in range(TILES):
                    nchunks = int(K_lo[t] + K_hi[t])
                    if nchunks == 0:
                        nc.vector.memset(target[:Fdim, _ts(t, 128)], 0.0)
                        continue
                    pscat = psc.tile([Fdim, 128], F32, tag="psc")
                    ci = 0
                    for rgn, Kr, Bs, off in ((0, K_lo, LB, 0),
                                             (1, K_hi, HB, C_lo)):
                        for k in range(int(Kr[t])):
                            ch = int(Bs[t]) + k          # global chunk id
                            rel = ch - off               # chunk id in region
                            g = ensure_wave(rgn, rel // WV)
                            pos = rel % WV
                            sw = ensure_s(rgn, rel // WVS)
                            spos = rel % WVS
                            nc.tensor.matmul(
                                pscat[:Fdim, :], g[:, pos, :Fdim],
                                sw[:, spos * 128:(spos + 1) * 128],
                                start=(ci == 0), stop=(ci == nchunks - 1))
                            ci += 1

                    # ---- 5. epilogue: *dinv[dst], +bias, ReLU ----
                    tmp = tmpp.tile([Fdim, 128], F32, tag="tmp")
                    nc.vector.scalar_tensor_tensor(
                        tmp[:Fdim, :], pscat[:Fdim, :], 0.0,
                        dinv[:Fdim, _ts(t, 128)],
                        mybir.AluOpType.bypass, mybir.AluOpType.mult)
                    if layer == 0:
                        nc.scalar.activation(
                            target[:Fdim, _ts(t, 128)], tmp[:Fdim, :],
                            mybir.ActivationFunctionType.Relu,
                            bias=bcol[:Fdim, :], scale=1.0)
                    else:
                        ot = outp.tile([OUT, 128], F32, tag="out")
                        nc.scalar.activation(
                            ot[:], tmp[:Fdim, :],
                            mybir.ActivationFunctionType.Relu,
                            bias=bcol[:Fdim, :], scale=1.0)
                        nc.sync.dma_start(out_d[:, _ts(t, 128)], ot[:])


    nc.compile()
    return nc


def _host_inputs(inputs, meta, cfg=None):
    g = cfg or default_cfg()
    N, IN, HID, OUT = g["N"], g["IN"], g["HID"], g["OUT"]
    NCORES, NPC_REAL, NPC = g["NCORES"], g["NPC_REAL"], g["NPC"]
    x = np.asarray(inputs["x"], np.float32)
    W1 = np.asarray(inputs["W1"], np.float32)
    b1 = np.asarray(inputs["b1"], np.float32)
    W2 = np.asarray(inputs["W2"], np.float32)
    b2 = np.asarray(inputs["b2"], np.float32)
    deg = meta["deg"]

    ident = np.eye(128, dtype=NPF16)
    w1c = W1.astype(NPF16)
    w2c = np.zeros((HID, OUT), NPF16)
    w2c[:, :] = W2.astype(NPF16)
    b1c = b1.reshape(HID, 1).astype(np.float32)
    b2c = b2.reshape(OUT, 1).astype(np.float32)

    in_maps = []
    for c in range(NCORES):
        xs = np.zeros((NPC, IN), np.float32)
        xs[:NPC_REAL] = x[c * NPC_REAL:(c + 1) * NPC_REAL]
        xT = np.ascontiguousarray(xs.T).astype(NPF16)

        node = np.arange(NPC) + c * NPC_REAL
        degs = np.ones(NPC, np.float32)
        degs[:NPC_REAL] = deg[node[:NPC_REAL]]
        degrep = np.tile(degs[None, :], (128, 1)).astype(np.float32)

        in_maps.append({
            "xT": xT, "W1": w1c, "W2": w2c, "b1c": b1c, "b2c": b2c,
            "degrep": degrep, "ident": ident,
            "idxt": meta["idx_t"][c], "sall": meta["sall"][c],
        })
    return in_maps


def kernel(**inputs) -> np.ndarray:
    meta = _preprocess(np.asarray(inputs["edge_index"]))
    nc = _build_program(meta)
    in_maps = _host_inputs(inputs, meta)
    res = run_bass_kernel_spmd(nc, in_maps, list(range(NCORES)))
    out = np.empty((N, OUT), np.float32)
    for c in range(NCORES):
        out[c * NPC_REAL:(c + 1) * NPC_REAL] = \
            res.results[c]["outT"][:, :NPC_REAL].T
    return out
